# revision 8
# baseline (speedup 1.0000x reference)
"""Binarize kernel for Trainium2: out[b, d, n/8] = packbits(x[b, :] > th[d]).

x: [2048, 32768] f32. depth_ths: [3] f32. out: [2048, 3, 4096] uint8.

Strategy (8-way data parallel over batch, 256 rows/core):
  - DMA x tiles [128, 8192] f32 into SBUF (double-buffered).
  - Compares: t0/t2 as is_gt on VectorE (fp8 {0,1} bits, 2x_2P mode),
    t1 as Sign on ScalarE (+-1 bits; byte = 0.5*S + 127.5 folds the
    {0,1} correction into the PSUM copy — requires no x == th exactly).
  - Bit packing on the PE with fp8 DoubleRow matmuls: bits viewed as
    [p, chunk, pair q, j, group] — each pair-matmul contracts the two
    ADJACENT bits (2q, 2q+1) with stacked scaled-identity weights
    (2^(7-2q), 2^(6-2q)), so 4 accumulating matmuls per output chunk
    instead of 8 (half the PE columns of the plain stride-8 scheme).
  - PSUM: one [128, 512] tile (one bank) per (plane, chunk); PSUM ->
    uint8 SBUF copies mostly on ScalarE (t1 via activation-Copy with
    scale=0.5 bias=127.5); on alternating tiles one copy goes to
    VectorE (dvc_sched=[1,0]) — ACT (Sign + copies) and DVE (2 is_gt)
    are within ~5% of each other, and the alternation measured ~5%
    faster sustained than either uniform assignment.
  - Output stores on the ScalarE HWDGE ring (loads on SyncE's), one
    flat contiguous 1.5 MiB store per 128-row block.
Per-tile engine budgets (burst regime): DMA ~7us, DVE ~8.6us
(2 is_gt), ACT ~9us (Sign + 6 copies), PE ~5.8us. Measured: ~139us/iter
sustained (k=202 in-program loop; engines downclock) vs ~206 for the
stride-8 single-bit scheme; ~66-79us/iter burst (k<=18) vs ~79-90.
"""

import sys

import numpy as np

try:
    from concourse import bacc, bass, mybir, tile
    from concourse.bass_utils import run_bass_kernel_spmd
except ImportError:  # fresh grading dir: concourse lives in the trn repo
    sys.path.insert(0, "/opt/trn_rl_repo")
    from concourse import bacc, bass, mybir, tile
    from concourse.bass_utils import run_bass_kernel_spmd

import ml_dtypes

B, N = 2048, 32768
NCORES = 8
ROWS = B // NCORES          # 256 rows per core
NB = N // 8                 # 4096 output bytes per row per threshold
P = 128                     # partitions
FT = 8192                   # free-dim tile of x (f32) per inner iteration
GT = FT // 8                # output bytes per x tile = 1024
CHUNK = 512                 # matmul free dim (half a PSUM plane tile)

_cache: dict = {}


def _build(
    ths: tuple[float, float, float],
    loop: int = 1,
    ft: int = FT,
    dvc_sched: tuple = (1, 0),  # per-tile count of PSUM copies routed to DVE
    store_engine: str = "scalar",
    xbufs: int = 2,
    bbufs: int = 4,
    psbufs: int = 6,
) -> "bass.Bass":
    nc = bacc.Bacc()
    x_in = nc.declare_dram_parameter("x", [ROWS, N], mybir.dt.float32, isOutput=False)
    w_in = nc.declare_dram_parameter(
        "w", [P, 8 * P], mybir.dt.float8e4, isOutput=False
    )
    out_ext = nc.declare_dram_parameter(
        "out", [ROWS, 3, NB], mybir.dt.uint8, isOutput=True
    )
    out_flat = out_ext.ap().rearrange("r d g -> r (d g)")  # [ROWS, 3*NB]
    gt = ft // 8
    nchunks = ft // (8 * CHUNK)
    assert nchunks == 2

    def body(tc, wv, xpool, bpool, opool, pspool):
        st = getattr(nc, store_engine)
        for pb in range(ROWS // P):          # 2 partition blocks
            r0 = pb * P
            ob = opool.tile([P, 3 * NB], mybir.dt.uint8)
            for fti in range(N // ft):       # free tiles
                c0 = fti * ft
                xt = xpool.tile([P, ft], mybir.dt.float32)
                nc.sync.dma_start(out=xt[:], in_=x_in[r0 : r0 + P, c0 : c0 + ft])
                bvs = []
                for t in range(3):
                    bits = bpool.tile(
                        [P, ft], mybir.dt.float8e4, name="bits", tag="bits"
                    )
                    if t == 1:
                        nc.scalar.activation(
                            out=bits[:], in_=xt[:],
                            func=mybir.ActivationFunctionType.Sign,
                            bias=-ths[t],
                        )
                    else:
                        nc.vector.tensor_scalar(
                            out=bits[:], in0=xt[:], scalar1=ths[t],
                            scalar2=None, op0=mybir.AluOpType.is_gt,
                        )
                    bvs.append(
                        bits.rearrange(
                            "p (c g e4 e1) -> p c e4 e1 g", g=CHUNK, e4=4, e1=2
                        )
                    )
                pss = {
                    (t, c): pspool.tile(
                        [P, CHUNK], mybir.dt.float32, name="ps", tag="ps"
                    )
                    for t in range(3)
                    for c in range(nchunks)
                }
                for q in range(4):
                    for t in range(3):
                        for c in range(nchunks):
                            nc.tensor.matmul(
                                pss[(t, c)][:],
                                wv[:, 2 * q : 2 * q + 2, :],
                                bvs[t][:, c, q, :, :],
                                start=(q == 0),
                                stop=(q == 3),
                                perf_mode=mybir.MatmulPerfMode.DoubleRow,
                            )
                ndv = 0
                tile_no = pb * (N // ft) + fti
                tile_dvc = dvc_sched[tile_no % len(dvc_sched)]
                order = [(1, c) for c in range(nchunks)] + [
                    (t, c) for t in (0, 2) for c in range(nchunks)
                ]
                for (t, c) in order:
                    ps = pss[(t, c)]
                    o0 = t * NB + fti * gt + c * CHUNK
                    oslice = ob[:, o0 : o0 + CHUNK]
                    on_dve = ndv < tile_dvc
                    ndv += 1
                    if t == 1:
                        if on_dve:
                            nc.vector.tensor_scalar(
                                out=oslice, in0=ps[:], scalar1=0.5, scalar2=127.5,
                                op0=mybir.AluOpType.mult, op1=mybir.AluOpType.add,
                            )
                        else:
                            nc.scalar.activation(
                                out=oslice, in_=ps[:],
                                func=mybir.ActivationFunctionType.Copy,
                                bias=127.5, scale=0.5,
                            )
                    else:
                        if on_dve:
                            nc.vector.tensor_copy(out=oslice, in_=ps[:])
                        else:
                            nc.scalar.copy(out=oslice, in_=ps[:])
            # one flat contiguous store per partition block (1.5 MiB)
            st.dma_start(out=out_flat[r0 : r0 + P, :], in_=ob[:])

    with tile.TileContext(nc) as tc:
        with (
            tc.tile_pool(name="wpool", bufs=1) as wpool,
            tc.tile_pool(name="xpool", bufs=xbufs) as xpool,
            tc.tile_pool(name="bpool", bufs=bbufs) as bpool,
            tc.tile_pool(name="opool", bufs=2) as opool,
            tc.tile_pool(name="psum", bufs=psbufs, space="PSUM") as pspool,
        ):
            wtile = wpool.tile([P, 8 * P], mybir.dt.float8e4)
            nc.sync.dma_start(out=wtile[:], in_=w_in[:])
            wv = wtile.rearrange("p (k m) -> p k m", k=8)
            if loop == 1:
                body(tc, wv, xpool, bpool, opool, pspool)
            else:
                with tc.For_i(0, loop, 1):
                    body(tc, wv, xpool, bpool, opool, pspool)
    nc.compile()
    return nc


def _weights() -> np.ndarray:
    dt = ml_dtypes.float8_e4m3fn
    w = np.zeros((P, 8 * P), dtype=dt)
    for i in range(8):
        np.fill_diagonal(w[:, i * P : (i + 1) * P], dt(2 ** (7 - i)))
    return w


def kernel(x: np.ndarray, depth_ths: np.ndarray) -> np.ndarray:
    x = np.asarray(x)
    ths = tuple(float(v) for v in np.asarray(depth_ths, dtype=np.float32))
    assert x.shape == (B, N) and len(ths) == 3

    if ths not in _cache:
        _cache[ths] = _build(ths)
    nc = _cache[ths]

    w = _weights()
    in_maps = [
        {"x": np.ascontiguousarray(x[i * ROWS : (i + 1) * ROWS]), "w": w}
        for i in range(NCORES)
    ]
    res = run_bass_kernel_spmd(nc, in_maps, list(range(NCORES)))
    return np.concatenate([res.results[i]["out"] for i in range(NCORES)], axis=0)


# revision 9
# speedup vs baseline: 2.2062x; 2.2062x over previous
"""Binarize kernel for Trainium2: out[b, d, n/8] = packbits(x[b, :] > th[d]).

x: [2048, 32768] f32. depth_ths: [3] f32. out: [2048, 3, 4096] uint8.

Strategy (8-way data parallel over batch, 256 rows/core):
  - DMA x tiles [128, 8192] f32 into SBUF (double-buffered).
  - Compares: t0/t2 as is_gt on VectorE (fp8 {0,1} bits, 2x_2P mode),
    t1 as Sign on ScalarE (+-1 bits; byte = 0.5*S + 127.5 folds the
    {0,1} correction into the PSUM copy — requires no x == th exactly).
  - Bit packing on the PE with fp8 DoubleRow matmuls: bits viewed as
    [p, chunk, pair q, j, group] — each pair-matmul contracts the two
    ADJACENT bits (2q, 2q+1) with stacked scaled-identity weights
    (2^(7-2q), 2^(6-2q)), so 4 accumulating matmuls per output chunk
    instead of 8 (half the PE columns of the plain stride-8 scheme).
  - PSUM: one [128, 512] tile (one bank) per (plane, chunk); PSUM ->
    uint8 SBUF copies mostly on ScalarE (t1 via activation-Copy with
    scale=0.5 bias=127.5); on alternating tiles one copy goes to
    VectorE (dvc_sched=[1,0]) — ACT (Sign + copies) and DVE (2 is_gt)
    are within ~5% of each other, and the alternation measured ~5%
    faster sustained than either uniform assignment.
  - Output stores on the ScalarE HWDGE ring (loads on SyncE's), one
    flat contiguous 1.5 MiB store per 128-row block.
Per-tile engine budgets (burst regime): DMA ~7us, DVE ~8.6us
(2 is_gt), ACT ~9us (Sign + 6 copies), PE ~5.8us. Measured: ~139us/iter
sustained (k=202 in-program loop; engines downclock) vs ~206 for the
stride-8 single-bit scheme; ~66-79us/iter burst (k<=18) vs ~79-90.
"""

import sys

import numpy as np

try:
    from concourse import bacc, bass, mybir, tile
    from concourse.bass_utils import run_bass_kernel_spmd
except ImportError:  # fresh grading dir: concourse lives in the trn repo
    sys.path.insert(0, "/opt/trn_rl_repo")
    from concourse import bacc, bass, mybir, tile
    from concourse.bass_utils import run_bass_kernel_spmd

import ml_dtypes

B, N = 2048, 32768
NCORES = 8
ROWS = B // NCORES          # 256 rows per core
NB = N // 8                 # 4096 output bytes per row per threshold
P = 128                     # partitions
FT = 8192                   # free-dim tile of x (f32) per inner iteration
GT = FT // 8                # output bytes per x tile = 1024
CHUNK = 512                 # matmul free dim (half a PSUM plane tile)

_cache: dict = {}


def _build(
    ths: tuple[float, float, float],
    loop: int = 1,
    ft: int = FT,
    dvc_sched: tuple = (1, 0),  # per-tile count of PSUM copies routed to DVE
    store_engine: str = "scalar",
    xbufs: int = 2,
    bbufs: int = 4,
    psbufs: int = 6,
) -> "bass.Bass":
    nc = bacc.Bacc()
    x_in = nc.declare_dram_parameter("x", [ROWS, N], mybir.dt.float32, isOutput=False)
    w_in = nc.declare_dram_parameter(
        "w", [P, 8 * P], mybir.dt.float8e4, isOutput=False
    )
    out_ext = nc.declare_dram_parameter(
        "out", [ROWS, 3, NB], mybir.dt.uint8, isOutput=True
    )
    out_flat = out_ext.ap().rearrange("r d g -> r (d g)")  # [ROWS, 3*NB]
    gt = ft // 8
    nchunks = ft // (8 * CHUNK)
    assert nchunks == 2

    def body(tc, wv, xpool, bpool, opool, pspool):
        st = getattr(nc, store_engine)
        for pb in range(ROWS // P):          # 2 partition blocks
            r0 = pb * P
            ob = opool.tile([P, 3 * NB], mybir.dt.uint8)
            for fti in range(N // ft):       # free tiles
                c0 = fti * ft
                xt = xpool.tile([P, ft], mybir.dt.float32)
                nc.sync.dma_start(out=xt[:], in_=x_in[r0 : r0 + P, c0 : c0 + ft])
                # compares at half-tile granularity, Sign issued first:
                # measured ~10% faster sustained than full-tile ops
                # (shorter ops pipeline against the PE and release xt
                # sooner; chunk c's matmuls only need half c's bits)
                bvs = [None, None, None]
                hw = ft // 2
                for t in (1, 0, 2):
                    bits = bpool.tile(
                        [P, ft], mybir.dt.float8e4, name="bits", tag="bits"
                    )
                    for h in range(2):
                        sl = slice(h * hw, (h + 1) * hw)
                        if t == 1:
                            nc.scalar.activation(
                                out=bits[:, sl], in_=xt[:, sl],
                                func=mybir.ActivationFunctionType.Sign,
                                bias=-ths[t],
                            )
                        else:
                            nc.vector.tensor_scalar(
                                out=bits[:, sl], in0=xt[:, sl], scalar1=ths[t],
                                scalar2=None, op0=mybir.AluOpType.is_gt,
                            )
                    bvs[t] = bits.rearrange(
                        "p (c g e4 e1) -> p c e4 e1 g", g=CHUNK, e4=4, e1=2
                    )
                pss = {
                    (t, c): pspool.tile(
                        [P, CHUNK], mybir.dt.float32, name="ps", tag="ps"
                    )
                    for t in range(3)
                    for c in range(nchunks)
                }
                for q in range(4):
                    for t in range(3):
                        for c in range(nchunks):
                            nc.tensor.matmul(
                                pss[(t, c)][:],
                                wv[:, 2 * q : 2 * q + 2, :],
                                bvs[t][:, c, q, :, :],
                                start=(q == 0),
                                stop=(q == 3),
                                perf_mode=mybir.MatmulPerfMode.DoubleRow,
                            )
                ndv = 0
                tile_no = pb * (N // ft) + fti
                tile_dvc = dvc_sched[tile_no % len(dvc_sched)]
                order = [(1, c) for c in range(nchunks)] + [
                    (t, c) for t in (0, 2) for c in range(nchunks)
                ]
                for (t, c) in order:
                    ps = pss[(t, c)]
                    o0 = t * NB + fti * gt + c * CHUNK
                    oslice = ob[:, o0 : o0 + CHUNK]
                    on_dve = ndv < tile_dvc
                    ndv += 1
                    if t == 1:
                        if on_dve:
                            nc.vector.tensor_scalar(
                                out=oslice, in0=ps[:], scalar1=0.5, scalar2=127.5,
                                op0=mybir.AluOpType.mult, op1=mybir.AluOpType.add,
                            )
                        else:
                            nc.scalar.activation(
                                out=oslice, in_=ps[:],
                                func=mybir.ActivationFunctionType.Copy,
                                bias=127.5, scale=0.5,
                            )
                    else:
                        if on_dve:
                            nc.vector.tensor_copy(out=oslice, in_=ps[:])
                        else:
                            nc.scalar.copy(out=oslice, in_=ps[:])
            # one flat contiguous store per partition block (1.5 MiB)
            st.dma_start(out=out_flat[r0 : r0 + P, :], in_=ob[:])

    with tile.TileContext(nc) as tc:
        with (
            tc.tile_pool(name="wpool", bufs=1) as wpool,
            tc.tile_pool(name="xpool", bufs=xbufs) as xpool,
            tc.tile_pool(name="bpool", bufs=bbufs) as bpool,
            tc.tile_pool(name="opool", bufs=2) as opool,
            tc.tile_pool(name="psum", bufs=psbufs, space="PSUM") as pspool,
        ):
            wtile = wpool.tile([P, 8 * P], mybir.dt.float8e4)
            nc.sync.dma_start(out=wtile[:], in_=w_in[:])
            wv = wtile.rearrange("p (k m) -> p k m", k=8)
            if loop == 1:
                body(tc, wv, xpool, bpool, opool, pspool)
            else:
                with tc.For_i(0, loop, 1):
                    body(tc, wv, xpool, bpool, opool, pspool)
    nc.compile()
    return nc


def _weights() -> np.ndarray:
    dt = ml_dtypes.float8_e4m3fn
    w = np.zeros((P, 8 * P), dtype=dt)
    for i in range(8):
        np.fill_diagonal(w[:, i * P : (i + 1) * P], dt(2 ** (7 - i)))
    return w


def kernel(x: np.ndarray, depth_ths: np.ndarray) -> np.ndarray:
    x = np.asarray(x)
    ths = tuple(float(v) for v in np.asarray(depth_ths, dtype=np.float32))
    assert x.shape == (B, N) and len(ths) == 3

    if ths not in _cache:
        _cache[ths] = _build(ths)
    nc = _cache[ths]

    w = _weights()
    in_maps = [
        {"x": np.ascontiguousarray(x[i * ROWS : (i + 1) * ROWS]), "w": w}
        for i in range(NCORES)
    ]
    res = run_bass_kernel_spmd(nc, in_maps, list(range(NCORES)))
    return np.concatenate([res.results[i]["out"] for i in range(NCORES)], axis=0)
